# revision 2
# baseline (speedup 1.0000x reference)
"""Causal single-head attention (shared-weight multi-head), 8-core Trainium2 Bass kernel.

v3: superpair structure. Per slot s the work is 4(s+1) "superpairs", each
covering one own-region 128-key chunk (tile row 0-63) and one other-region
chunk (tile row 64-127):
  2 score matmuls (K=64 contract) -> one [128,1024] PSUM pair (2 banks)
  1 exp over [qlo:1024) (ACT, scale=1/8) -> pt [128,1024] bf16
  2 PV matmuls (V1 [128,65] stationary) accumulating into outT [65,512]
Projections: [Wq|Wq] dup -> QT2 (moving for both row tiles); own blocks
[Wk|Wv], other blocks [Wv|Wk] -> K^T/V^T halves recombined by DVE into
KT2 (K own top / K other bottom) and VT2 (V other top / V own bottom); one
[128,128] PE transpose then yields BOTH blocks' token-major V chunks, stored
with ones columns as V2 [128,4,130]. Diagonal chunks stream only the causal
query suffix; pad-block invalidation via a 0/1 scalar multiply on pt.
Normalization (col 64 of outT = denominator) + block permutation + x16 head
replication happen on host.
"""

import os
import numpy as np
import ml_dtypes

B, T, E, HEAD, NH = 4, 4096, 1024, 64, 16
BLK = 512
NCORES = 8
KE = E // 128
OWN = {0: [0, 3, 4, 7], 1: [1, 2, 5, 6]}
PADMASK = {0: [0.0, 1.0, 0.0, 1.0], 1: [1.0, 0.0, 1.0, 0.0]}

_prog_cache = {}


def _build_program(reps=None):
    import concourse.bass as bass
    import concourse.mybir as mybir
    import concourse.tile as tile
    from concourse import bacc
    import contextlib

    f32 = mybir.dt.float32
    bf16 = mybir.dt.bfloat16
    EXP = mybir.ActivationFunctionType.Exp

    nc = bacc.Bacc("TRN2", target_bir_lowering=False, debug=False, num_devices=NCORES)

    xT = nc.dram_tensor("xT", [E, T], bf16, kind="ExternalInput").ap()
    wqq = nc.dram_tensor("wqq", [E, 128], bf16, kind="ExternalInput").ap()
    wkvt = nc.dram_tensor("wkvt", [E, 128], bf16, kind="ExternalInput").ap()
    wkvb = nc.dram_tensor("wkvb", [E, 128], bf16, kind="ExternalInput").ap()
    ident = nc.dram_tensor("ident", [128, 128], bf16, kind="ExternalInput").ap()
    tri = nc.dram_tensor("tri", [128, 128], bf16, kind="ExternalInput").ap()
    pmask = nc.dram_tensor("pmask", [128, 4], f32, kind="ExternalInput").ap()
    out = nc.dram_tensor("out", [65, 4, BLK], f32, kind="ExternalOutput").ap()

    # X_OUTSIDE=1: hoist the x DMAs out of the measurement loop (diagnostic
    # only — separates input-DMA serialization from compute in looped timing).
    x_outside = False

    with tile.TileContext(nc) as tc:
        with tc.tile_pool(name="outer", bufs=1) as outer:
            # loop-invariant weight/const loads (pre-loop; in the single-shot
            # program this region is simply the program head)
            wqq_sb = outer.tile([128, KE, 128], bf16)
            wkvt_sb = outer.tile([128, KE, 128], bf16)
            wkvb_sb = outer.tile([128, KE, 128], bf16)
            nc.sync.dma_start(out=wqq_sb,
                              in_=wqq.rearrange("(k p) d -> p k d", p=128))
            nc.sync.dma_start(out=wkvt_sb,
                              in_=wkvt.rearrange("(k p) d -> p k d", p=128))
            nc.sync.dma_start(out=wkvb_sb,
                              in_=wkvb.rearrange("(k p) d -> p k d", p=128))
            id_sb = outer.tile([128, 128], bf16)
            tri_sb = outer.tile([128, 128], bf16)
            pm_sb = outer.tile([128, 4], f32)
            nc.sync.dma_start(out=id_sb, in_=ident)
            nc.sync.dma_start(out=tri_sb, in_=tri)
            nc.sync.dma_start(out=pm_sb, in_=pmask)
            xo_sb = None
            if x_outside:
                xo_sb = outer.tile([128, KE, 8, BLK], bf16)
                xTr0 = xT.rearrange("(k p) t -> p k t", p=128)
                for pos in range(8):
                    nc.sync.dma_start(
                        out=xo_sb[:, :, pos, :],
                        in_=xTr0[:, :, pos * BLK:(pos + 1) * BLK],
                    )
            loop_ctx = (tc.For_i(0, reps, 1,
                                 hint_engines=(mybir.EngineType.PE,))
                        if reps else contextlib.nullcontext())
            with (
                loop_ctx,
                tc.tile_pool(name="singles", bufs=1) as singles,
                tc.tile_pool(
                    name="xpool",
                    bufs=1) as xpool,
                tc.tile_pool(name="psum_proj", bufs=2, space="PSUM") as psum_proj,
                tc.tile_pool(name="psum_s", bufs=2, space="PSUM") as psum_s,
                tc.tile_pool(name="psum_o", bufs=2, space="PSUM") as psum_o,
                tc.tile_pool(name="ptil", bufs=3) as ptil_pool,
                tc.tile_pool(name="work", bufs=4) as work,
            ):
                # ---- x loads ----
                # Host orders positions interleaved (own0, oth0, own1, oth1,
                # ...) so wave w's two blocks are columns [2w*512, 2w*512+1024)
                # and a k-chunk DMA slice covers whole waves with one
                # contiguous line per partition.
                if x_outside:
                    x_sb = xo_sb
                else:
                    x_sb = xpool.tile([128, KE, 8, BLK], bf16, tag="x")
                    xTr = xT.rearrange("(k p) t -> p k t", p=128)
                    for wp in range(2):  # wave pairs: waves 0-1, then 2-3
                        for k in range(KE):
                            eng = nc.sync if (k % 2 == 0) else nc.scalar
                            eng.dma_start(
                                out=x_sb[:, k, 4 * wp:4 * wp + 4, :],
                                in_=xTr[:, k, wp * 2048:(wp + 1) * 2048],
                            )

                QT2 = [singles.tile([128, BLK], bf16, name=f"qt2_{s}")
                       for s in range(4)]
                KT2 = [singles.tile([128, BLK], bf16, name=f"kt2_{w}")
                       for w in range(4)]
                VT2 = [singles.tile([128, BLK], bf16, name=f"vt2_{w}")
                       for w in range(4)]
                V2 = [singles.tile([128, 4, 130], bf16, name=f"v2_{w}")
                      for w in range(4)]

                def proj(w_sb, pos):
                    ps = psum_proj.tile([128, BLK], f32, tag="proj")
                    for k in range(KE):
                        nc.tensor.matmul(
                            ps, w_sb[:, k, :], x_sb[:, k, pos, :],
                            start=(k == 0), stop=(k == KE - 1),
                        )
                    return ps

                def wave(w):
                    ps = proj(wqq_sb, 2 * w)
                    nc.vector.tensor_copy(QT2[w], ps)
                    ps = proj(wkvt_sb, 2 * w)    # own: K top, V bottom
                    nc.vector.tensor_copy(KT2[w][0:64, :], ps[0:64, :])
                    nc.vector.tensor_copy(VT2[w][64:128, :], ps[64:128, :])
                    ps = proj(wkvb_sb, 2 * w + 1)  # other: V top, K bottom
                    nc.vector.tensor_copy(KT2[w][64:128, :], ps[64:128, :])
                    nc.vector.tensor_copy(VT2[w][0:64, :], ps[0:64, :])
                    for c in range(4):
                        tp = psum_proj.tile([128, 128], bf16, tag="proj")
                        nc.tensor.transpose(
                            tp, VT2[w][:, c * 128:(c + 1) * 128], id_sb)
                        # cols 0:64 -> other-block V tokens, 64:128 -> own
                        nc.vector.tensor_copy(V2[w][:, c, 0:64], tp[:, 0:64])
                        nc.vector.tensor_copy(V2[w][:, c, 65:129],
                                              tp[:, 64:128])
                    nc.vector.memset(V2[w][:, :, 64:65], 1.0)
                    nc.vector.memset(V2[w][:, :, 129:130], 1.0)

                wave(0)
                for s in range(4):
                    # own half: diag chunks (wave s, restricted) first, then
                    # full own blocks (waves 0..s-1); other half: waves 0..s,
                    # position 4+w, wave s's other block being pad-masked.
                    own = [(s, c, c * 128) for c in range(4)]
                    own += [(p, c, 0) for p in range(s) for c in range(4)]
                    oth = [(w, c) for w in range(s + 1) for c in range(4)]
                    npair = len(own)
                    o_ps = psum_o.tile([128, BLK], f32, tag="o")
                    for i in range(npair):
                        wo, co, qlo = own[i]
                        wx, cx = oth[i]
                        s_ps = psum_s.tile([128, 2 * BLK], f32, tag="s")
                        nc.tensor.matmul(
                            s_ps[:, qlo:BLK],
                            KT2[wo][0:64, co * 128:(co + 1) * 128],
                            QT2[s][0:64, qlo:BLK],
                            start=True, stop=True,
                        )
                        nc.tensor.matmul(
                            s_ps[:, BLK:2 * BLK],
                            KT2[wx][64:128, cx * 128:(cx + 1) * 128],
                            QT2[s][64:128, :],
                            start=True, stop=True,
                        )
                        pt = ptil_pool.tile([128, 2 * BLK], bf16, tag="pt")
                        nc.scalar.activation(
                            pt[:, qlo:2 * BLK], s_ps[:, qlo:2 * BLK], EXP,
                            scale=0.125,
                        )
                        if i < 4:  # diag chunk: triangular mask
                            nc.vector.tensor_mul(
                                pt[:, qlo:qlo + 128], pt[:, qlo:qlo + 128],
                                tri_sb,
                            )
                        if wx == s:  # newest other block: 0/1 validity mask
                            nc.vector.tensor_scalar_mul(
                                pt[:, BLK:2 * BLK], pt[:, BLK:2 * BLK],
                                pm_sb[:, s:s + 1],
                            )
                        nc.tensor.matmul(
                            o_ps[0:65, qlo:BLK], V2[wo][:, co, 65:130],
                            pt[:, qlo:BLK],
                            start=(i == 0), stop=False,
                        )
                        nc.tensor.matmul(
                            o_ps[0:65, :], V2[wx][:, cx, 0:65],
                            pt[:, BLK:2 * BLK],
                            start=False, stop=(i == npair - 1),
                        )
                    o_sb = work.tile([128, BLK], f32, tag="osb")
                    nc.vector.tensor_copy(o_sb[0:65, :], o_ps[0:65, :])
                    nc.scalar.dma_start(out=out[:, s, :], in_=o_sb[0:65, :])
                    if s < 3:
                        wave(s + 1)

    nc.compile()
    return nc


def _host_inputs(embedded, Wq, Wk, Wv):
    """Per-core input maps (host does layout only: transpose/permute/cast)."""
    bf = ml_dtypes.bfloat16
    emb = np.asarray(embedded, dtype=np.float32)
    wq = np.asarray(Wq, dtype=np.float32)
    wk = np.asarray(Wk, dtype=np.float32)
    wv = np.asarray(Wv, dtype=np.float32)

    wqq = np.concatenate([wq, wq], axis=1).astype(bf)
    wkvt = np.concatenate([wk, wv], axis=1).astype(bf)
    wkvb = np.concatenate([wv, wk], axis=1).astype(bf)
    ident = np.eye(128, dtype=np.float32).astype(bf)
    p = np.arange(128)[:, None]
    f = np.arange(128)[None, :]
    tri = (p <= f).astype(bf)

    in_maps = []
    for b in range(B):
        for role in range(2):
            order = [b for pair in zip(OWN[role], OWN[1 - role])
                     for b in pair]
            xTb = emb[b].T  # [E, T]
            xTp = np.concatenate(
                [xTb[:, j * BLK:(j + 1) * BLK] for j in order], axis=1
            ).astype(bf)
            pm = np.broadcast_to(
                np.asarray(PADMASK[role], np.float32), (128, 4))
            in_maps.append({
                "xT": np.ascontiguousarray(xTp),
                "wqq": wqq, "wkvt": wkvt, "wkvb": wkvb,
                "ident": ident, "tri": np.ascontiguousarray(tri),
                "pmask": np.ascontiguousarray(pm),
            })
    return in_maps


def _run(nc, in_maps, trace=False):
    from concourse.bass_utils import run_bass_kernel_spmd
    return run_bass_kernel_spmd(nc, in_maps, list(range(NCORES)), trace=trace)


def _assemble(results):
    head = np.empty((B, T, HEAD), dtype=np.float32)
    for core, r in enumerate(results):
        b, role = divmod(core, 2)
        o = np.asarray(r["out"])  # [65, 4, 512]
        for s in range(4):
            j = OWN[role][s]
            blkT = o[0:HEAD, s, :] / o[HEAD, s, :]
            head[b, j * BLK:(j + 1) * BLK, :] = blkT.T
    return np.tile(head, (1, 1, NH))


def kernel(embedded, Wq, Wk, Wv, num_heads):
    num_heads = int(num_heads)
    assert num_heads == NH

    if "nc" not in _prog_cache:
        _prog_cache["nc"] = _build_program()
    nc = _prog_cache["nc"]

    in_maps = _host_inputs(embedded, Wq, Wk, Wv)
    res = _run(nc, in_maps, trace=bool(int(os.environ.get("KERNEL_TRACE", "0"))))
    _prog_cache["last_result"] = res
    return _assemble(res.results)


# revision 3
# speedup vs baseline: 1.0117x; 1.0117x over previous
"""Causal single-head attention (shared-weight multi-head), 8-core Trainium2 Bass kernel.

v3: superpair structure. Per slot s the work is 4(s+1) "superpairs", each
covering one own-region 128-key chunk (tile row 0-63) and one other-region
chunk (tile row 64-127):
  2 score matmuls (K=64 contract) -> one [128,1024] PSUM pair (2 banks)
  1 exp over [qlo:1024) (ACT, scale=1/8) -> pt [128,1024] bf16
  2 PV matmuls (V1 [128,65] stationary) accumulating into outT [65,512]
Projections: [Wq|Wq] dup -> QT2 (moving for both row tiles); own blocks
[Wk|Wv], other blocks [Wv|Wk] -> K^T/V^T halves recombined by DVE into
KT2 (K own top / K other bottom) and VT2 (V other top / V own bottom); one
[128,128] PE transpose then yields BOTH blocks' token-major V chunks, stored
with ones columns as V2 [128,4,130]. Diagonal chunks stream only the causal
query suffix; pad-block invalidation via a 0/1 scalar multiply on pt.
Normalization (col 64 of outT = denominator) + block permutation + x16 head
replication happen on host.
"""

import os
import numpy as np
import ml_dtypes

B, T, E, HEAD, NH = 4, 4096, 1024, 64, 16
BLK = 512
NCORES = 8
KE = E // 128
OWN = {0: [0, 3, 4, 7], 1: [1, 2, 5, 6]}
PADMASK = {0: [0.0, 1.0, 0.0, 1.0], 1: [1.0, 0.0, 1.0, 0.0]}

_prog_cache = {}


def _build_program(reps=None):
    import concourse.bass as bass
    import concourse.mybir as mybir
    import concourse.tile as tile
    from concourse import bacc
    import contextlib

    f32 = mybir.dt.float32
    bf16 = mybir.dt.bfloat16
    EXP = mybir.ActivationFunctionType.Exp

    nc = bacc.Bacc("TRN2", target_bir_lowering=False, debug=False, num_devices=NCORES)

    xT = nc.dram_tensor("xT", [E, T], bf16, kind="ExternalInput").ap()
    wqq = nc.dram_tensor("wqq", [E, 128], bf16, kind="ExternalInput").ap()
    wkvt = nc.dram_tensor("wkvt", [E, 128], bf16, kind="ExternalInput").ap()
    wkvb = nc.dram_tensor("wkvb", [E, 128], bf16, kind="ExternalInput").ap()
    ident = nc.dram_tensor("ident", [128, 128], bf16, kind="ExternalInput").ap()
    tri = nc.dram_tensor("tri", [128, 128], bf16, kind="ExternalInput").ap()
    pmask = nc.dram_tensor("pmask", [128, 4], f32, kind="ExternalInput").ap()
    out = nc.dram_tensor("out", [65, 4, BLK], f32, kind="ExternalOutput").ap()

    # X_OUTSIDE=1: hoist the x DMAs out of the measurement loop (diagnostic
    # only — separates input-DMA serialization from compute in looped timing).
    x_outside = False

    with tile.TileContext(nc) as tc:
        with tc.tile_pool(name="outer", bufs=1) as outer:
            # loop-invariant weight/const loads (pre-loop; in the single-shot
            # program this region is simply the program head)
            wqq_sb = outer.tile([128, KE, 128], bf16)
            wkvt_sb = outer.tile([128, KE, 128], bf16)
            wkvb_sb = outer.tile([128, KE, 128], bf16)
            nc.sync.dma_start(out=wqq_sb,
                              in_=wqq.rearrange("(k p) d -> p k d", p=128))
            nc.scalar.dma_start(out=wkvt_sb,
                                in_=wkvt.rearrange("(k p) d -> p k d", p=128))
            nc.scalar.dma_start(out=wkvb_sb,
                                in_=wkvb.rearrange("(k p) d -> p k d", p=128))
            id_sb = outer.tile([128, 128], bf16)
            tri_sb = outer.tile([128, 128], bf16)
            pm_sb = outer.tile([128, 4], f32)
            nc.scalar.dma_start(out=id_sb, in_=ident)
            nc.scalar.dma_start(out=tri_sb, in_=tri)
            nc.scalar.dma_start(out=pm_sb, in_=pmask)
            xo_sb = None
            if x_outside:
                xo_sb = outer.tile([128, KE, 8, BLK], bf16)
                xTr0 = xT.rearrange("(k p) t -> p k t", p=128)
                for pos in range(8):
                    nc.sync.dma_start(
                        out=xo_sb[:, :, pos, :],
                        in_=xTr0[:, :, pos * BLK:(pos + 1) * BLK],
                    )
            loop_ctx = (tc.For_i(0, reps, 1,
                                 hint_engines=(mybir.EngineType.PE,))
                        if reps else contextlib.nullcontext())
            with (
                loop_ctx,
                tc.tile_pool(name="singles", bufs=1) as singles,
                tc.tile_pool(name="xpool", bufs=1) as xpool,
                tc.tile_pool(name="psum_proj", bufs=3, space="PSUM") as psum_proj,
                tc.tile_pool(name="psum_s", bufs=2, space="PSUM") as psum_s,
                tc.tile_pool(name="psum_o", bufs=1, space="PSUM") as psum_o,
                tc.tile_pool(name="ptil", bufs=4) as ptil_pool,
                tc.tile_pool(name="work", bufs=4) as work,
            ):
                # ---- x loads ----
                # Host orders positions interleaved (own0, oth0, own1, oth1,
                # ...) so wave w's two blocks are columns [2w*512, 2w*512+1024)
                # and a k-chunk DMA slice covers whole waves with one
                # contiguous line per partition.
                if x_outside:
                    x_sb = xo_sb
                else:
                    x_sb = xpool.tile([128, KE, 8, BLK], bf16, tag="x")
                    xTr = xT.rearrange("(k p) t -> p k t", p=128)
                    for w in range(4):  # per-wave k-chunks, 2 queues
                        for k in range(KE):
                            eng = nc.sync if (k % 2 == 0) else nc.scalar
                            eng.dma_start(
                                out=x_sb[:, k, 2 * w:2 * w + 2, :],
                                in_=xTr[:, k, w * 1024:(w + 1) * 1024],
                            )

                QT2 = [singles.tile([128, BLK], bf16, name=f"qt2_{s}")
                       for s in range(4)]
                KT2 = [singles.tile([128, BLK], bf16, name=f"kt2_{w}")
                       for w in range(4)]
                VT2 = [singles.tile([128, BLK], bf16, name=f"vt2_{w}")
                       for w in range(4)]
                V2 = [singles.tile([128, 4, 130], bf16, name=f"v2_{w}")
                      for w in range(4)]

                def proj(w_sb, pos):
                    ps = psum_proj.tile([128, BLK], f32, tag="proj")
                    for k in range(KE):
                        nc.tensor.matmul(
                            ps, w_sb[:, k, :], x_sb[:, k, pos, :],
                            start=(k == 0), stop=(k == KE - 1),
                        )
                    return ps

                def wave(w):
                    ps = proj(wqq_sb, 2 * w)
                    nc.vector.tensor_copy(QT2[w], ps)
                    ps = proj(wkvt_sb, 2 * w)    # own: K top, V bottom
                    nc.vector.tensor_copy(KT2[w][0:64, :], ps[0:64, :])
                    nc.vector.tensor_copy(VT2[w][64:128, :], ps[64:128, :])
                    ps = proj(wkvb_sb, 2 * w + 1)  # other: V top, K bottom
                    nc.vector.tensor_copy(KT2[w][64:128, :], ps[64:128, :])
                    nc.vector.tensor_copy(VT2[w][0:64, :], ps[0:64, :])
                    for c in range(4):
                        tp = psum_proj.tile([128, 128], bf16, tag="proj")
                        nc.tensor.transpose(
                            tp, VT2[w][:, c * 128:(c + 1) * 128], id_sb)
                        # cols 0:64 -> other-block V tokens, 64:128 -> own
                        nc.vector.tensor_copy(V2[w][:, c, 0:64], tp[:, 0:64])
                        nc.vector.tensor_copy(V2[w][:, c, 65:129],
                                              tp[:, 64:128])
                    nc.vector.memset(V2[w][:, :, 64:65], 1.0)
                    nc.vector.memset(V2[w][:, :, 129:130], 1.0)

                wave(0)
                for s in range(4):
                    # own half: diag chunks (wave s, restricted) first, then
                    # full own blocks (waves 0..s-1); other half: waves 0..s,
                    # position 4+w, wave s's other block being pad-masked.
                    own = [(s, c, c * 128) for c in range(4)]
                    own += [(p, c, 0) for p in range(s) for c in range(4)]
                    oth = [(w, c) for w in range(s + 1) for c in range(4)]
                    npair = len(own)
                    o_ps = psum_o.tile([128, BLK], f32, tag="o")
                    for i in range(npair):
                        wo, co, qlo = own[i]
                        wx, cx = oth[i]
                        s_ps = psum_s.tile([128, 2 * BLK], f32, tag="s")
                        nc.tensor.matmul(
                            s_ps[:, qlo:BLK],
                            KT2[wo][0:64, co * 128:(co + 1) * 128],
                            QT2[s][0:64, qlo:BLK],
                            start=True, stop=True,
                        )
                        nc.tensor.matmul(
                            s_ps[:, BLK:2 * BLK],
                            KT2[wx][64:128, cx * 128:(cx + 1) * 128],
                            QT2[s][64:128, :],
                            start=True, stop=True,
                        )
                        pt = ptil_pool.tile([128, 2 * BLK], bf16, tag="pt")
                        nc.scalar.activation(
                            pt[:, qlo:2 * BLK], s_ps[:, qlo:2 * BLK], EXP,
                            scale=0.125,
                        )
                        if i < 4:  # diag chunk: triangular mask
                            nc.vector.tensor_mul(
                                pt[:, qlo:qlo + 128], pt[:, qlo:qlo + 128],
                                tri_sb,
                            )
                        if wx == s:  # newest other block: 0/1 validity mask
                            nc.vector.tensor_scalar_mul(
                                pt[:, BLK:2 * BLK], pt[:, BLK:2 * BLK],
                                pm_sb[:, s:s + 1],
                            )
                        nc.tensor.matmul(
                            o_ps[0:65, qlo:BLK], V2[wo][:, co, 65:130],
                            pt[:, qlo:BLK],
                            start=(i == 0), stop=False,
                        )
                        nc.tensor.matmul(
                            o_ps[0:65, :], V2[wx][:, cx, 0:65],
                            pt[:, BLK:2 * BLK],
                            start=False, stop=(i == npair - 1),
                        )
                    o_sb = work.tile([128, BLK], f32, tag="osb")
                    nc.vector.tensor_copy(o_sb[0:65, :], o_ps[0:65, :])
                    nc.scalar.dma_start(out=out[:, s, :], in_=o_sb[0:65, :])
                    if s < 3:
                        wave(s + 1)

    nc.compile()
    return nc


def _host_inputs(embedded, Wq, Wk, Wv):
    """Per-core input maps (host does layout only: transpose/permute/cast)."""
    bf = ml_dtypes.bfloat16
    emb = np.asarray(embedded, dtype=np.float32)
    wq = np.asarray(Wq, dtype=np.float32)
    wk = np.asarray(Wk, dtype=np.float32)
    wv = np.asarray(Wv, dtype=np.float32)

    wqq = np.concatenate([wq, wq], axis=1).astype(bf)
    wkvt = np.concatenate([wk, wv], axis=1).astype(bf)
    wkvb = np.concatenate([wv, wk], axis=1).astype(bf)
    ident = np.eye(128, dtype=np.float32).astype(bf)
    p = np.arange(128)[:, None]
    f = np.arange(128)[None, :]
    tri = (p <= f).astype(bf)

    in_maps = []
    for b in range(B):
        for role in range(2):
            order = [b for pair in zip(OWN[role], OWN[1 - role])
                     for b in pair]
            xTb = emb[b].T  # [E, T]
            xTp = np.concatenate(
                [xTb[:, j * BLK:(j + 1) * BLK] for j in order], axis=1
            ).astype(bf)
            pm = np.broadcast_to(
                np.asarray(PADMASK[role], np.float32), (128, 4))
            in_maps.append({
                "xT": np.ascontiguousarray(xTp),
                "wqq": wqq, "wkvt": wkvt, "wkvb": wkvb,
                "ident": ident, "tri": np.ascontiguousarray(tri),
                "pmask": np.ascontiguousarray(pm),
            })
    return in_maps


def _run(nc, in_maps, trace=False):
    from concourse.bass_utils import run_bass_kernel_spmd
    return run_bass_kernel_spmd(nc, in_maps, list(range(NCORES)), trace=trace)


def _assemble(results):
    head = np.empty((B, T, HEAD), dtype=np.float32)
    for core, r in enumerate(results):
        b, role = divmod(core, 2)
        o = np.asarray(r["out"])  # [65, 4, 512]
        for s in range(4):
            j = OWN[role][s]
            blkT = o[0:HEAD, s, :] / o[HEAD, s, :]
            head[b, j * BLK:(j + 1) * BLK, :] = blkT.T
    return np.tile(head, (1, 1, NH))


def kernel(embedded, Wq, Wk, Wv, num_heads):
    num_heads = int(num_heads)
    assert num_heads == NH

    if "nc" not in _prog_cache:
        _prog_cache["nc"] = _build_program()
    nc = _prog_cache["nc"]

    in_maps = _host_inputs(embedded, Wq, Wk, Wv)
    res = _run(nc, in_maps, trace=bool(int(os.environ.get("KERNEL_TRACE", "0"))))
    _prog_cache["last_result"] = res
    return _assemble(res.results)


# revision 4
# speedup vs baseline: 1.0438x; 1.0318x over previous
"""Causal single-head attention (shared-weight multi-head), 8-core Trainium2 Bass kernel.

v3: superpair structure. Per slot s the work is 4(s+1) "superpairs", each
covering one own-region 128-key chunk (tile row 0-63) and one other-region
chunk (tile row 64-127):
  2 score matmuls (K=64 contract) -> one [128,1024] PSUM pair (2 banks)
  1 exp over [qlo:1024) (ACT, scale=1/8) -> pt [128,1024] bf16
  2 PV matmuls (V1 [128,65] stationary) accumulating into outT [65,512]
Projections: [Wq|Wq] dup -> QT2 (moving for both row tiles); own blocks
[Wk|Wv], other blocks [Wv|Wk] -> K^T/V^T halves recombined by DVE into
KT2 (K own top / K other bottom) and VT2 (V other top / V own bottom); one
[128,128] PE transpose then yields BOTH blocks' token-major V chunks, stored
with ones columns as V2 [128,4,130]. Diagonal chunks stream only the causal
query suffix; pad-block invalidation via a 0/1 scalar multiply on pt.
Normalization (col 64 of outT = denominator) + block permutation + x16 head
replication happen on host.
"""

import os
import numpy as np
import ml_dtypes

B, T, E, HEAD, NH = 4, 4096, 1024, 64, 16
BLK = 512
NCORES = 8
KE = E // 128
OWN = {0: [0, 3, 4, 7], 1: [1, 2, 5, 6]}
PADMASK = {0: [0.0, 1.0, 0.0, 1.0], 1: [1.0, 0.0, 1.0, 0.0]}

_prog_cache = {}


def _build_program(reps=None):
    import concourse.bass as bass
    import concourse.mybir as mybir
    import concourse.tile as tile
    from concourse import bacc
    import contextlib

    f32 = mybir.dt.float32
    bf16 = mybir.dt.bfloat16
    EXP = mybir.ActivationFunctionType.Exp

    nc = bacc.Bacc("TRN2", target_bir_lowering=False, debug=False, num_devices=NCORES)

    xT = nc.dram_tensor("xT", [E, T], bf16, kind="ExternalInput").ap()
    wqq = nc.dram_tensor("wqq", [E, 128], bf16, kind="ExternalInput").ap()
    wkvt = nc.dram_tensor("wkvt", [E, 128], bf16, kind="ExternalInput").ap()
    wkvb = nc.dram_tensor("wkvb", [E, 128], bf16, kind="ExternalInput").ap()
    ident = nc.dram_tensor("ident", [128, 128], bf16, kind="ExternalInput").ap()
    tri = nc.dram_tensor("tri", [128, 128], bf16, kind="ExternalInput").ap()
    pmask = nc.dram_tensor("pmask", [128, 4], f32, kind="ExternalInput").ap()
    out = nc.dram_tensor("out", [65, 4, BLK], f32, kind="ExternalOutput").ap()

    # X_OUTSIDE=1: hoist the x DMAs out of the measurement loop (diagnostic
    # only — separates input-DMA serialization from compute in looped timing).
    x_outside = False

    with tile.TileContext(nc) as tc:
        with tc.tile_pool(name="outer", bufs=1) as outer:
            # loop-invariant weight/const loads (pre-loop; in the single-shot
            # program this region is simply the program head)
            wqq_sb = outer.tile([128, KE, 128], bf16)
            wkvt_sb = outer.tile([128, KE, 128], bf16)
            wkvb_sb = outer.tile([128, KE, 128], bf16)
            nc.sync.dma_start(out=wqq_sb,
                              in_=wqq.rearrange("(k p) d -> p k d", p=128))
            nc.scalar.dma_start(out=wkvt_sb,
                                in_=wkvt.rearrange("(k p) d -> p k d", p=128))
            nc.scalar.dma_start(out=wkvb_sb,
                                in_=wkvb.rearrange("(k p) d -> p k d", p=128))
            id_sb = outer.tile([128, 128], bf16)
            tri_sb = outer.tile([128, 128], bf16)
            pm_sb = outer.tile([128, 4], f32)
            nc.scalar.dma_start(out=id_sb, in_=ident)
            nc.scalar.dma_start(out=tri_sb, in_=tri)
            nc.scalar.dma_start(out=pm_sb, in_=pmask)
            xo_sb = None
            if x_outside:
                xo_sb = outer.tile([128, KE, 8, BLK], bf16)
                xTr0 = xT.rearrange("(k p) t -> p k t", p=128)
                for pos in range(8):
                    nc.sync.dma_start(
                        out=xo_sb[:, :, pos, :],
                        in_=xTr0[:, :, pos * BLK:(pos + 1) * BLK],
                    )
            loop_ctx = (tc.For_i(0, reps, 1,
                                 hint_engines=(mybir.EngineType.PE,))
                        if reps else contextlib.nullcontext())
            with (
                loop_ctx,
                tc.tile_pool(name="singles", bufs=1) as singles,
                tc.tile_pool(name="xpool", bufs=1) as xpool,
                tc.tile_pool(name="psum_proj", bufs=3, space="PSUM") as psum_proj,
                tc.tile_pool(name="psum_s", bufs=2, space="PSUM") as psum_s,
                tc.tile_pool(name="psum_o", bufs=1, space="PSUM") as psum_o,
                tc.tile_pool(name="ptil", bufs=4) as ptil_pool,
                tc.tile_pool(name="work", bufs=4) as work,
            ):
                # ---- x loads ----
                # Host orders positions interleaved (own0, oth0, own1, oth1,
                # ...) so wave w's two blocks are columns [2w*512, 2w*512+1024)
                # and a k-chunk DMA slice covers whole waves with one
                # contiguous line per partition.
                if x_outside:
                    x_sb = xo_sb
                else:
                    x_sb = xpool.tile([128, KE, 8, BLK], bf16, tag="x")
                    xTr = xT.rearrange("(k p) t -> p k t", p=128)
                    for w in range(4):  # per-wave k-chunks, 2 queues
                        for k in range(KE):
                            eng = nc.sync if (k % 2 == 0) else nc.scalar
                            eng.dma_start(
                                out=x_sb[:, k, 2 * w:2 * w + 2, :],
                                in_=xTr[:, k, w * 1024:(w + 1) * 1024],
                            )

                QT2 = [singles.tile([128, BLK], bf16, name=f"qt2_{s}")
                       for s in range(4)]
                KT2 = [singles.tile([128, BLK], bf16, name=f"kt2_{w}")
                       for w in range(4)]
                VT2 = [singles.tile([128, BLK], bf16, name=f"vt2_{w}")
                       for w in range(4)]
                V2 = [singles.tile([128, 4, 130], bf16, name=f"v2_{w}")
                      for w in range(4)]

                def proj(w_sb, pos):
                    ps = psum_proj.tile([128, BLK], f32, tag="proj")
                    for k in range(KE):
                        nc.tensor.matmul(
                            ps, w_sb[:, k, :], x_sb[:, k, pos, :],
                            start=(k == 0), stop=(k == KE - 1),
                        )
                    return ps

                def wave(w):
                    ps = proj(wqq_sb, 2 * w)
                    nc.vector.tensor_copy(QT2[w], ps)
                    ps = proj(wkvt_sb, 2 * w)    # own: K top, V bottom
                    nc.vector.tensor_copy(KT2[w][0:64, :], ps[0:64, :])
                    nc.vector.tensor_copy(VT2[w][64:128, :], ps[64:128, :])
                    ps = proj(wkvb_sb, 2 * w + 1)  # other: V top, K bottom
                    nc.vector.tensor_copy(KT2[w][64:128, :], ps[64:128, :])
                    nc.vector.tensor_copy(VT2[w][0:64, :], ps[0:64, :])
                    for c in range(4):
                        tp = psum_proj.tile([128, 128], bf16, tag="proj")
                        nc.tensor.transpose(
                            tp, VT2[w][:, c * 128:(c + 1) * 128], id_sb)
                        # cols 0:64 -> other-block V tokens, 64:128 -> own
                        nc.vector.tensor_copy(V2[w][:, c, 0:64], tp[:, 0:64])
                        nc.vector.tensor_copy(V2[w][:, c, 65:129],
                                              tp[:, 64:128])
                    nc.vector.memset(V2[w][:, :, 64:65], 1.0)
                    nc.vector.memset(V2[w][:, :, 129:130], 1.0)

                wave(0)
                for s in range(4):
                    # own half: diag chunks (wave s, restricted) first, then
                    # full own blocks (waves 0..s-1); other half: waves 0..s,
                    # position 4+w, wave s's other block being pad-masked.
                    own = [(s, c, c * 128) for c in range(4)]
                    own += [(p, c, 0) for p in range(s) for c in range(4)]
                    oth = [(w, c) for w in range(s + 1) for c in range(4)]
                    npair = len(own)
                    o_ps = psum_o.tile([128, BLK], f32, tag="o")

                    def emit_front(i):
                        wo, co, qlo = own[i]
                        wx, cx = oth[i]
                        s_ps = psum_s.tile([128, 2 * BLK], f32, tag="s")
                        nc.tensor.matmul(
                            s_ps[:, qlo:BLK],
                            KT2[wo][0:64, co * 128:(co + 1) * 128],
                            QT2[s][0:64, qlo:BLK],
                            start=True, stop=True,
                        )
                        nc.tensor.matmul(
                            s_ps[:, BLK:2 * BLK],
                            KT2[wx][64:128, cx * 128:(cx + 1) * 128],
                            QT2[s][64:128, :],
                            start=True, stop=True,
                        )
                        pt = ptil_pool.tile([128, 2 * BLK], bf16, tag="pt")
                        nc.scalar.activation(
                            pt[:, qlo:2 * BLK], s_ps[:, qlo:2 * BLK], EXP,
                            scale=0.125,
                        )
                        if i < 4:  # diag chunk: triangular mask
                            nc.vector.tensor_mul(
                                pt[:, qlo:qlo + 128], pt[:, qlo:qlo + 128],
                                tri_sb,
                            )
                        if wx == s:  # newest other block: 0/1 validity mask
                            nc.vector.tensor_scalar_mul(
                                pt[:, BLK:2 * BLK], pt[:, BLK:2 * BLK],
                                pm_sb[:, s:s + 1],
                            )
                        return pt

                    def emit_pv(i, pt):
                        wo, co, qlo = own[i]
                        wx, cx = oth[i]
                        nc.tensor.matmul(
                            o_ps[0:65, qlo:BLK], V2[wo][:, co, 65:130],
                            pt[:, qlo:BLK],
                            start=(i == 0), stop=False,
                        )
                        nc.tensor.matmul(
                            o_ps[0:65, :], V2[wx][:, cx, 0:65],
                            pt[:, BLK:2 * BLK],
                            start=False, stop=(i == npair - 1),
                        )

                    prev = None
                    for i in range(npair):
                        pt = emit_front(i)
                        if prev is not None:
                            emit_pv(i - 1, prev)
                        prev = pt
                    emit_pv(npair - 1, prev)
                    o_sb = work.tile([128, BLK], f32, tag="osb")
                    nc.vector.tensor_copy(o_sb[0:65, :], o_ps[0:65, :])
                    nc.scalar.dma_start(out=out[:, s, :], in_=o_sb[0:65, :])
                    if s < 3:
                        wave(s + 1)

    nc.compile()
    return nc


def _host_inputs(embedded, Wq, Wk, Wv):
    """Per-core input maps (host does layout only: transpose/permute/cast)."""
    bf = ml_dtypes.bfloat16
    emb = np.asarray(embedded, dtype=np.float32)
    wq = np.asarray(Wq, dtype=np.float32)
    wk = np.asarray(Wk, dtype=np.float32)
    wv = np.asarray(Wv, dtype=np.float32)

    wqq = np.concatenate([wq, wq], axis=1).astype(bf)
    wkvt = np.concatenate([wk, wv], axis=1).astype(bf)
    wkvb = np.concatenate([wv, wk], axis=1).astype(bf)
    ident = np.eye(128, dtype=np.float32).astype(bf)
    p = np.arange(128)[:, None]
    f = np.arange(128)[None, :]
    tri = (p <= f).astype(bf)

    in_maps = []
    for b in range(B):
        for role in range(2):
            order = [b for pair in zip(OWN[role], OWN[1 - role])
                     for b in pair]
            xTb = emb[b].T  # [E, T]
            xTp = np.concatenate(
                [xTb[:, j * BLK:(j + 1) * BLK] for j in order], axis=1
            ).astype(bf)
            pm = np.broadcast_to(
                np.asarray(PADMASK[role], np.float32), (128, 4))
            in_maps.append({
                "xT": np.ascontiguousarray(xTp),
                "wqq": wqq, "wkvt": wkvt, "wkvb": wkvb,
                "ident": ident, "tri": np.ascontiguousarray(tri),
                "pmask": np.ascontiguousarray(pm),
            })
    return in_maps


def _run(nc, in_maps, trace=False):
    from concourse.bass_utils import run_bass_kernel_spmd
    return run_bass_kernel_spmd(nc, in_maps, list(range(NCORES)), trace=trace)


def _assemble(results):
    head = np.empty((B, T, HEAD), dtype=np.float32)
    for core, r in enumerate(results):
        b, role = divmod(core, 2)
        o = np.asarray(r["out"])  # [65, 4, 512]
        for s in range(4):
            j = OWN[role][s]
            blkT = o[0:HEAD, s, :] / o[HEAD, s, :]
            head[b, j * BLK:(j + 1) * BLK, :] = blkT.T
    return np.tile(head, (1, 1, NH))


def kernel(embedded, Wq, Wk, Wv, num_heads):
    num_heads = int(num_heads)
    assert num_heads == NH

    if "nc" not in _prog_cache:
        _prog_cache["nc"] = _build_program()
    nc = _prog_cache["nc"]

    in_maps = _host_inputs(embedded, Wq, Wk, Wv)
    res = _run(nc, in_maps, trace=bool(int(os.environ.get("KERNEL_TRACE", "0"))))
    _prog_cache["last_result"] = res
    return _assemble(res.results)


# revision 5
# speedup vs baseline: 1.1129x; 1.0661x over previous
"""Causal single-head attention (shared-weight multi-head), 8-core Trainium2 Bass kernel.

v3: superpair structure. Per slot s the work is 4(s+1) "superpairs", each
covering one own-region 128-key chunk (tile row 0-63) and one other-region
chunk (tile row 64-127):
  2 score matmuls (K=64 contract) -> one [128,1024] PSUM pair (2 banks)
  1 exp over [qlo:1024) (ACT, scale=1/8) -> pt [128,1024] bf16
  2 PV matmuls (V1 [128,65] stationary) accumulating into outT [65,512]
Projections: [Wq|Wq] dup -> QT2 (moving for both row tiles); own blocks
[Wk|Wv], other blocks [Wv|Wk] -> K^T/V^T halves recombined by DVE into
KT2 (K own top / K other bottom) and VT2 (V other top / V own bottom); one
[128,128] PE transpose then yields BOTH blocks' token-major V chunks, stored
with ones columns as V2 [128,4,130]. Diagonal chunks stream only the causal
query suffix; pad-block invalidation via a 0/1 scalar multiply on pt.
Normalization (col 64 of outT = denominator) + block permutation + x16 head
replication happen on host.
"""

import os
import numpy as np
import ml_dtypes

B, T, E, HEAD, NH = 4, 4096, 1024, 64, 16
BLK = 512
NCORES = 8
KE = E // 128
OWN = {0: [0, 3, 4, 7], 1: [1, 2, 5, 6]}
PADMASK = {0: [0.0, 1.0, 0.0, 1.0], 1: [1.0, 0.0, 1.0, 0.0]}

_prog_cache = {}


def _build_program(reps=None):
    import concourse.bass as bass
    import concourse.mybir as mybir
    import concourse.tile as tile
    from concourse import bacc
    import contextlib

    f32 = mybir.dt.float32
    bf16 = mybir.dt.bfloat16
    EXP = mybir.ActivationFunctionType.Exp

    nc = bacc.Bacc("TRN2", target_bir_lowering=False, debug=False, num_devices=NCORES)

    xT = nc.dram_tensor("xT", [E, T], bf16, kind="ExternalInput").ap()
    wqq = nc.dram_tensor("wqq", [E, 128], bf16, kind="ExternalInput").ap()
    wkvt = nc.dram_tensor("wkvt", [E, 128], bf16, kind="ExternalInput").ap()
    wkvb = nc.dram_tensor("wkvb", [E, 128], bf16, kind="ExternalInput").ap()
    ident = nc.dram_tensor("ident", [128, 128], bf16, kind="ExternalInput").ap()
    tri = nc.dram_tensor("tri", [128, 128], bf16, kind="ExternalInput").ap()
    pmask = nc.dram_tensor("pmask", [128, 4], f32, kind="ExternalInput").ap()
    out = nc.dram_tensor("out", [65, 4, BLK], f32, kind="ExternalOutput").ap()

    # X_OUTSIDE=1: hoist the x DMAs out of the measurement loop (diagnostic
    # only — separates input-DMA serialization from compute in looped timing).
    x_outside = False

    with tile.TileContext(nc) as tc:
        with tc.tile_pool(name="outer", bufs=1) as outer:
            # loop-invariant weight/const loads (pre-loop; in the single-shot
            # program this region is simply the program head)
            wqq_sb = outer.tile([128, KE, 128], bf16)
            wkvt_sb = outer.tile([128, KE, 128], bf16)
            wkvb_sb = outer.tile([128, KE, 128], bf16)
            nc.sync.dma_start(out=wqq_sb,
                              in_=wqq.rearrange("(k p) d -> p k d", p=128))
            nc.scalar.dma_start(out=wkvt_sb,
                                in_=wkvt.rearrange("(k p) d -> p k d", p=128))
            nc.scalar.dma_start(out=wkvb_sb,
                                in_=wkvb.rearrange("(k p) d -> p k d", p=128))
            id_sb = outer.tile([128, 128], bf16)
            tri_sb = outer.tile([128, 128], bf16)
            pm_sb = outer.tile([128, 4], f32)
            nc.scalar.dma_start(out=id_sb, in_=ident)
            nc.scalar.dma_start(out=tri_sb, in_=tri)
            nc.scalar.dma_start(out=pm_sb, in_=pmask)
            xo_sb = None
            if x_outside:
                xo_sb = outer.tile([128, KE, 8, BLK], bf16)
                xTr0 = xT.rearrange("(k p) t -> p k t", p=128)
                for pos in range(8):
                    nc.sync.dma_start(
                        out=xo_sb[:, :, pos, :],
                        in_=xTr0[:, :, pos * BLK:(pos + 1) * BLK],
                    )
            loop_ctx = (tc.For_i(0, reps, 1,
                                 hint_engines=(mybir.EngineType.PE,))
                        if reps else contextlib.nullcontext())
            with (
                loop_ctx,
                tc.tile_pool(name="singles", bufs=1) as singles,
                tc.tile_pool(name="xpool", bufs=1) as xpool,
                tc.tile_pool(name="psum_proj", bufs=3, space="PSUM") as psum_proj,
                tc.tile_pool(name="psum_s", bufs=2, space="PSUM") as psum_s,
                tc.tile_pool(name="psum_o", bufs=1, space="PSUM") as psum_o,
                tc.tile_pool(name="ptil", bufs=4) as ptil_pool,
                tc.tile_pool(name="work", bufs=4) as work,
            ):
                # ---- x loads ----
                # Host orders positions interleaved (own0, oth0, own1, oth1,
                # ...) so wave w's two blocks are columns [2w*512, 2w*512+1024)
                # and a k-chunk DMA slice covers whole waves with one
                # contiguous line per partition.
                if x_outside:
                    x_sb = xo_sb
                else:
                    x_sb = xpool.tile([128, KE, 8, BLK], bf16, tag="x")
                    xTr = xT.rearrange("(k p) t -> p k t", p=128)
                    for w in range(4):  # per-wave k-chunks, 2 queues
                        for k in range(KE):
                            eng = nc.sync if (k % 2 == 0) else nc.scalar
                            eng.dma_start(
                                out=x_sb[:, k, 2 * w:2 * w + 2, :],
                                in_=xTr[:, k, w * 1024:(w + 1) * 1024],
                            )

                QT2 = [singles.tile([128, BLK], bf16, name=f"qt2_{s}")
                       for s in range(4)]
                KT2 = [singles.tile([128, BLK], bf16, name=f"kt2_{w}")
                       for w in range(4)]
                VT2 = [singles.tile([128, BLK], bf16, name=f"vt2_{w}")
                       for w in range(4)]
                V2 = [singles.tile([128, 4, 130], bf16, name=f"v2_{w}")
                      for w in range(4)]
                V2p = [singles.tile([128, 4, 65], bf16, name=f"v2p_{w}")
                       for w in range(4)]

                def proj(w_sb, pos):
                    ps = psum_proj.tile([128, BLK], f32, tag="proj")
                    for k in range(KE):
                        nc.tensor.matmul(
                            ps, w_sb[:, k, :], x_sb[:, k, pos, :],
                            start=(k == 0), stop=(k == KE - 1),
                        )
                    return ps

                def wave(w):
                    ps = proj(wqq_sb, 2 * w)
                    nc.vector.tensor_copy(QT2[w], ps)
                    ps = proj(wkvt_sb, 2 * w)    # own: K top, V bottom
                    nc.vector.tensor_copy(KT2[w][0:64, :], ps[0:64, :])
                    nc.vector.tensor_copy(VT2[w][64:128, :], ps[64:128, :])
                    ps = proj(wkvb_sb, 2 * w + 1)  # other: V top, K bottom
                    nc.vector.tensor_copy(KT2[w][64:128, :], ps[64:128, :])
                    nc.vector.tensor_copy(VT2[w][0:64, :], ps[0:64, :])
                    for c in range(4):
                        tp = psum_proj.tile([128, 128], bf16, tag="proj")
                        nc.tensor.transpose(
                            tp, VT2[w][:, c * 128:(c + 1) * 128], id_sb)
                        # cols 0:64 -> other-block V tokens, 64:128 -> own
                        nc.vector.tensor_copy(V2[w][:, c, 0:64], tp[:, 0:64])
                        nc.vector.tensor_copy(V2[w][:, c, 65:129],
                                              tp[:, 64:128])
                    nc.vector.memset(V2[w][:, :, 64:65], 1.0)
                    nc.vector.memset(V2[w][:, :, 129:130], 1.0)
                    # pre-masked copy of the other-block [V|1] for the one
                    # slot (s == w) where this block's validity is data
                    nc.vector.tensor_scalar_mul(
                        V2p[w], V2[w][:, :, 0:65], pm_sb[:, w:w + 1])

                wave(0)
                for s in range(4):
                    # own half: diag chunks (wave s, restricted) first, then
                    # full own blocks (waves 0..s-1); other half: waves 0..s,
                    # position 4+w, wave s's other block being pad-masked.
                    own = [(s, c, c * 128) for c in range(4)]
                    own += [(p, c, 0) for p in range(s) for c in range(4)]
                    oth = [(w, c) for w in range(s + 1) for c in range(4)]
                    npair = len(own)
                    o_ps = psum_o.tile([128, BLK], f32, tag="o")

                    def emit_front(i):
                        wo, co, qlo = own[i]
                        wx, cx = oth[i]
                        s_ps = psum_s.tile([128, 2 * BLK], f32, tag="s")
                        nc.tensor.matmul(
                            s_ps[:, qlo:BLK],
                            KT2[wo][0:64, co * 128:(co + 1) * 128],
                            QT2[s][0:64, qlo:BLK],
                            start=True, stop=True,
                        )
                        nc.tensor.matmul(
                            s_ps[:, BLK:2 * BLK],
                            KT2[wx][64:128, cx * 128:(cx + 1) * 128],
                            QT2[s][64:128, :],
                            start=True, stop=True,
                        )
                        pt = ptil_pool.tile([128, 2 * BLK], bf16, tag="pt")
                        nc.scalar.activation(
                            pt[:, qlo:2 * BLK], s_ps[:, qlo:2 * BLK], EXP,
                            scale=0.125,
                        )
                        if i < 4:  # diag chunk: triangular mask
                            nc.vector.tensor_mul(
                                pt[:, qlo:qlo + 128], pt[:, qlo:qlo + 128],
                                tri_sb,
                            )
                        return pt

                    def emit_pv(i, pt):
                        wo, co, qlo = own[i]
                        wx, cx = oth[i]
                        nc.tensor.matmul(
                            o_ps[0:65, qlo:BLK], V2[wo][:, co, 65:130],
                            pt[:, qlo:BLK],
                            start=(i == 0), stop=False,
                        )
                        vb = (V2p[wx][:, cx, :] if wx == s
                              else V2[wx][:, cx, 0:65])
                        nc.tensor.matmul(
                            o_ps[0:65, :], vb,
                            pt[:, BLK:2 * BLK],
                            start=False, stop=(i == npair - 1),
                        )

                    prev = None
                    for i in range(npair):
                        pt = emit_front(i)
                        if prev is not None:
                            emit_pv(i - 1, prev)
                        prev = pt
                    emit_pv(npair - 1, prev)
                    o_sb = work.tile([128, BLK], f32, tag="osb")
                    nc.vector.tensor_copy(o_sb[0:65, :], o_ps[0:65, :])
                    nc.scalar.dma_start(out=out[:, s, :], in_=o_sb[0:65, :])
                    if s < 3:
                        wave(s + 1)

    nc.compile()
    return nc


def _host_inputs(embedded, Wq, Wk, Wv):
    """Per-core input maps (host does layout only: transpose/permute/cast)."""
    bf = ml_dtypes.bfloat16
    emb = np.asarray(embedded, dtype=np.float32)
    wq = np.asarray(Wq, dtype=np.float32)
    wk = np.asarray(Wk, dtype=np.float32)
    wv = np.asarray(Wv, dtype=np.float32)

    wqq = np.concatenate([wq, wq], axis=1).astype(bf)
    wkvt = np.concatenate([wk, wv], axis=1).astype(bf)
    wkvb = np.concatenate([wv, wk], axis=1).astype(bf)
    ident = np.eye(128, dtype=np.float32).astype(bf)
    p = np.arange(128)[:, None]
    f = np.arange(128)[None, :]
    tri = (p <= f).astype(bf)

    in_maps = []
    for b in range(B):
        for role in range(2):
            order = [b for pair in zip(OWN[role], OWN[1 - role])
                     for b in pair]
            xTb = emb[b].T  # [E, T]
            xTp = np.concatenate(
                [xTb[:, j * BLK:(j + 1) * BLK] for j in order], axis=1
            ).astype(bf)
            pm = np.broadcast_to(
                np.asarray(PADMASK[role], np.float32), (128, 4))
            in_maps.append({
                "xT": np.ascontiguousarray(xTp),
                "wqq": wqq, "wkvt": wkvt, "wkvb": wkvb,
                "ident": ident, "tri": np.ascontiguousarray(tri),
                "pmask": np.ascontiguousarray(pm),
            })
    return in_maps


def _run(nc, in_maps, trace=False):
    from concourse.bass_utils import run_bass_kernel_spmd
    return run_bass_kernel_spmd(nc, in_maps, list(range(NCORES)), trace=trace)


def _assemble(results):
    head = np.empty((B, T, HEAD), dtype=np.float32)
    for core, r in enumerate(results):
        b, role = divmod(core, 2)
        o = np.asarray(r["out"])  # [65, 4, 512]
        for s in range(4):
            j = OWN[role][s]
            blkT = o[0:HEAD, s, :] / o[HEAD, s, :]
            head[b, j * BLK:(j + 1) * BLK, :] = blkT.T
    return np.tile(head, (1, 1, NH))


def kernel(embedded, Wq, Wk, Wv, num_heads):
    num_heads = int(num_heads)
    assert num_heads == NH

    if "nc" not in _prog_cache:
        _prog_cache["nc"] = _build_program()
    nc = _prog_cache["nc"]

    in_maps = _host_inputs(embedded, Wq, Wk, Wv)
    res = _run(nc, in_maps, trace=bool(int(os.environ.get("KERNEL_TRACE", "0"))))
    _prog_cache["last_result"] = res
    return _assemble(res.results)
